# revision 22
# baseline (speedup 1.0000x reference)
"""Causal GQA attention (B=2, S=2048, HID=2048, H=16, KVH=4, D=128) on 8 TRN2 cores.

Sharding: core c -> batch c//4, kv-group c%4 (4 q-heads + 1 kv-head).
o_proj is row-split by head group; host sums the 4 partials per batch.

Device kernel (per core, fp16 matmuls / f32 accumulation), fully streamed:
the sequence is processed in four 512-token quarters. Per quarter: v/k
projections (ko-interleaved so PE starts as soon as each xT chunk lands),
q projections, then attention for that query quarter (kv blocks 0..end of
quarter) and o_proj for the previous quarter are interleaved between later
projections so the PE never sits behind a DMA or phase barrier.

Softmax denominators are NOT computed with ones-matmuls on the PE (that
doubles attention PE traffic); instead the exp tiles are accumulated
elementwise on the DVE (fp16 atsum) and a single [1,512] ones-matmul per
(head, quarter) reduces across partitions.
"""

import numpy as np
import ml_dtypes

F16 = np.float16

B, S, HID = 2, 2048, 2048
H, KVH, D = 16, 4, 128
P = 128
KO = HID // P          # 16 contraction chunks of 128
HQ = H // KVH          # 4 q heads per core
NQ = 4                 # token quarters
QT = S // NQ           # 512 tokens per quarter
NKB = S // P           # 16 key blocks of 128
N_CORES = 8

_CACHE = {}


def _build_nc():
    import concourse.tile as tile
    from concourse import bacc, mybir
    from concourse.masks import make_identity
    import concourse.bass_isa as bass_isa
    from contextlib import ExitStack

    f16 = mybir.dt.float16
    f32 = mybir.dt.float32
    AF = mybir.ActivationFunctionType

    nc = bacc.Bacc("TRN2", target_bir_lowering=False, debug=False,
                   num_devices=N_CORES)

    xT_d = nc.dram_tensor("xT", [NQ * 4 * P, 4 * QT], f16, kind="ExternalInput").ap()
    wq_d = nc.dram_tensor("wq", [P, HQ * KO * D], f16, kind="ExternalInput").ap()
    wk_d = nc.dram_tensor("wk", [P, KO * D], f16, kind="ExternalInput").ap()
    wv_d = nc.dram_tensor("wv", [P, KO * D], f16, kind="ExternalInput").ap()
    wo_d = nc.dram_tensor("wo", [P, HQ * HID], f16, kind="ExternalInput").ap()
    cs_d = nc.dram_tensor("cs2", [P, S], f16, kind="ExternalInput").ap()
    ss_d = nc.dram_tensor("ss2", [P, S], f16, kind="ExternalInput").ap()
    mk_d = nc.dram_tensor("mask", [P, P], f16, kind="ExternalInput").ap()
    out_d = nc.dram_tensor("out", [S, HID], f16, kind="ExternalOutput").ap()
    out_r = out_d.rearrange("(tb p) o -> p tb o", p=P)

    with tile.TileContext(nc) as tc:
        with ExitStack() as octx:
            const = octx.enter_context(tc.tile_pool(name="const", bufs=1))
            rope_p = octx.enter_context(tc.tile_pool(name="rope", bufs=3))
            at_p = octx.enter_context(tc.tile_pool(name="at", bufs=5))
            asum_p = octx.enter_context(tc.tile_pool(name="asum", bufs=2))
            small = octx.enter_context(tc.tile_pool(name="small", bufs=3))
            ost_p = octx.enter_context(tc.tile_pool(name="ost", bufs=4))
            xt_ctx = ExitStack()
            xt_pool = xt_ctx.enter_context(tc.tile_pool(name="xt", bufs=1))

            # ---- persistent input DMAs, emitted in consumption order.
            # The DMA queues are descriptor-rate-bound (~4KB/partition per
            # descriptor is the sweet spot) and one dma_start lands on one
            # queue, so every big tensor is packed into 4KB-per-partition
            # runs and split into partition-halves to parallelize across
            # queues. ----
            def dmaN(dst, src, n):
                step = P // n
                for i in range(n):
                    nc.sync.dma_start(dst[i * step:(i + 1) * step],
                                      src[i * step:(i + 1) * step])

            def dma2(dst, src):
                dmaN(dst, src, 2)

            # first-needed tensors in partition-quarters (4 queues each)
            wk_sb = const.tile([P, KO, D], f16, tag="wk", name="wk")
            dmaN(wk_sb, wk_d.rearrange("p (ko n) -> p ko n", ko=KO), 4)
            wv_sb = const.tile([P, KO, D], f16, tag="wv", name="wv")
            dmaN(wv_sb, wv_d.rearrange("p (ko n) -> p ko n", ko=KO), 4)

            wq_sb = const.tile([P, HQ, KO, D], f16, tag="wq", name="wq")
            wq_r = wq_d.rearrange("p (h ko n) -> p h ko n", h=HQ, ko=KO)
            dmaN(wq_sb[:, 0], wq_r[:, 0], 4)

            # xT arrives as groups of 4 ko-chunks (4KB/partition per DMA)
            xTg = [[None] * 4 for _ in range(NQ)]

            def load_xq(tq, n=2):
                for kg in range(4):
                    t = xt_pool.tile([P, 4, QT], f16, tag=f"xT{tq}_{kg}",
                                     name=f"xT{tq}_{kg}")
                    base = (tq * 4 + kg) * P
                    dmaN(t.rearrange("p a b -> p (a b)"),
                         xT_d[base:base + P, :], n)
                    xTg[tq][kg] = t

            def xchunk(tq, ko):
                return xTg[tq][ko // 4][:, ko % 4, :]

            load_xq(0, n=4)

            for h in range(1, HQ):
                dma2(wq_sb[:, h], wq_r[:, h])

            cs_sb = const.tile([P, S], f16, tag="cs", name="cs")
            dma2(cs_sb, cs_d)
            ss_sb = const.tile([P, S], f16, tag="ss", name="ss")
            dma2(ss_sb, ss_d)
            mk_sb = const.tile([P, P], f16, tag="mk", name="mk")
            nc.sync.dma_start(mk_sb[:], mk_d[:])

            for tq in range(1, NQ):
                load_xq(tq)

            wo_sb = const.tile([P, HQ, HID], f16, tag="wo", name="wo")
            wo_r = wo_d.rearrange("p (h o) -> p h o", h=HQ)
            for h in range(HQ):
                dma2(wo_sb[:, h], wo_r[:, h])

            ident = const.tile([P, P], f16, tag="ident", name="ident")
            make_identity(nc, ident[:])
            ones = const.tile([P, 1], f16, tag="ones", name="ones")
            nc.vector.memset(ones[:], 1.0)

            qR = [const.tile([P, S], f16, tag=f"qR{h}", name=f"qR{h}")
                  for h in range(HQ)]
            kR = const.tile([P, S], f16, tag="kR", name="kR")
            vN = const.tile([P, NKB, D], f16, tag="vN", name="vN")
            avN = [const.tile([P, S], f16, tag=f"avN{h}", name=f"avN{h}")
                   for h in range(HQ)]

            with ExitStack() as ctx:
                # PSUM: proj(2) + s(2) + av(2) + oproj(2) = 8 banks
                proj_pp = ctx.enter_context(
                    tc.tile_pool(name="ps_proj", bufs=2, space="PSUM"))
                s_pp = ctx.enter_context(
                    tc.tile_pool(name="ps_s", bufs=2, space="PSUM"))
                av_pp = ctx.enter_context(
                    tc.tile_pool(name="ps_av", bufs=2, space="PSUM"))
                o_pp = ctx.enter_context(
                    tc.tile_pool(name="ps_o", bufs=2, space="PSUM"))

                # rotate partitions by 64 (stream_shuffle moves 4-partition
                # groups); keeps the rope half-swap off the DMA queues
                SWAP64 = list(range(16, 32)) + list(range(16))

                def rope_tile(ps, out_sl, tsl):
                    raw = rope_p.tile([P, QT], f16, tag="rp_raw", name="rp_raw")
                    nc.scalar.copy(raw[:], ps[:])
                    sw = rope_p.tile([P, QT], f16, tag="rp_sw", name="rp_sw")
                    nc.vector.stream_shuffle(sw[:], raw[:], SWAP64)
                    t1 = rope_p.tile([P, QT], f16, tag="rp_t1", name="rp_t1")
                    nc.vector.tensor_mul(t1[:], raw[:], cs_sb[:, tsl])
                    t2 = rope_p.tile([P, QT], f16, tag="rp_t2", name="rp_t2")
                    nc.vector.tensor_mul(t2[:], sw[:], ss_sb[:, tsl])
                    nc.vector.tensor_add(out_sl, t1[:], t2[:])

                def proj_kq0(tq):
                    """k and q-head-0 projections, ko-interleaved so the PE
                    starts on each xT chunk as soon as its DMA lands. Each
                    GEMM accumulates in its own PSUM bank — hardware PSUM
                    accumulation breaks if two start=True groups share a
                    bank."""
                    k_ps = proj_pp.tile([P, QT], f32, tag="proj", name="k_ps")
                    q_ps = proj_pp.tile([P, QT], f32, tag="proj", name="q_ps")
                    for ko in range(KO):
                        st, sp = ko == 0, ko == KO - 1
                        x = xchunk(tq, ko)
                        nc.tensor.matmul(k_ps[:], lhsT=wk_sb[:, ko, :],
                                         rhs=x, start=st, stop=sp)
                        nc.tensor.matmul(q_ps[:], lhsT=wq_sb[:, 0, ko, :],
                                         rhs=x, start=st, stop=sp)
                    tsl = slice(tq * QT, (tq + 1) * QT)
                    rope_tile(k_ps, kR[:, tsl], tsl)
                    rope_tile(q_ps, qR[0][:, tsl], tsl)

                def proj_v(tq):
                    """v projection straight to natural [tok, d] layout:
                    lhsT = xT chunk block, rhs = wv. The four 128-token
                    blocks run sequentially, each in its own psum ring slot
                    (one open accumulation group per bank)."""
                    for i in range(4):
                        v_ps = proj_pp.tile([P, P], f32, tag="proj",
                                            name="v_ps")
                        for ko in range(KO):
                            nc.tensor.matmul(
                                v_ps[:],
                                lhsT=xchunk(tq, ko)[:, i * P:(i + 1) * P],
                                rhs=wv_sb[:, ko, :],
                                start=(ko == 0), stop=(ko == KO - 1))
                        nc.scalar.copy(vN[:, 4 * tq + i, :], v_ps[:])

                def proj_q(tq, h):
                    q_ps = proj_pp.tile([P, QT], f32, tag="proj", name="q_ps")
                    for ko in range(KO):
                        nc.tensor.matmul(
                            q_ps[:], lhsT=wq_sb[:, h, ko, :],
                            rhs=xchunk(tq, ko),
                            start=(ko == 0), stop=(ko == KO - 1))
                    tsl = slice(tq * QT, (tq + 1) * QT)
                    rope_tile(q_ps, qR[h][:, tsl], tsl)

                # ---- attention for query quarter tq, head h ----
                # per-kb: scores -> exp -> (tri-mask) -> {AV matmul, atsum}
                # issue scores[i+1] before AV[i] so the PE always has work
                # while the ScalarE exps the previous block.
                ast = {}

                def attn_scores(tq, h, kb):
                    nkb = 4 * tq + 4
                    q0 = tq * QT
                    if kb == 0:
                        ast[(tq, h)] = {
                            "av": av_pp.tile([P, QT], f32, tag="av", name="av"),
                            "asum": asum_p.tile([P, QT], f16, tag="asum",
                                                name="asum"),
                            "ats": {},
                        }
                    st = ast[(tq, h)]
                    r = kb - 4 * tq
                    lo = P * r if r >= 0 else 0
                    s_ps = s_pp.tile([P, QT], f32, tag="s", name="s")
                    diag = r >= 0
                    nc.tensor.matmul(
                        s_ps[:, lo:QT],
                        lhsT=kR[:, kb * P:(kb + 1) * P],
                        rhs=qR[h][:, q0 + lo:q0 + QT],
                        start=True, stop=not diag)
                    if diag:
                        # push masked entries to -inf inside the psum group:
                        # exp then underflows to 0, no DVE hop before AV
                        nc.tensor.matmul(
                            s_ps[:, lo:lo + P], lhsT=ident[:], rhs=mk_sb[:],
                            start=False, stop=True)
                    at = at_p.tile([P, QT], f16, tag="at", name="at")
                    nc.scalar.activation(at[:, lo:QT], s_ps[:, lo:QT], AF.Exp)
                    if kb == 0:
                        nc.vector.tensor_copy(st["asum"][:], at[:])
                    else:
                        nc.vector.tensor_add(st["asum"][:, lo:QT],
                                             st["asum"][:, lo:QT], at[:, lo:QT])
                    st["ats"][kb] = at

                def attn_av(tq, h, kb):
                    nkb = 4 * tq + 4
                    q0 = tq * QT
                    st = ast[(tq, h)]
                    r = kb - 4 * tq
                    lo = P * r if r >= 0 else 0
                    at = st["ats"].pop(kb)
                    nc.tensor.matmul(
                        st["av"][:, lo:QT], lhsT=vN[:, kb, :],
                        rhs=at[:, lo:QT],
                        start=(kb == 0), stop=(kb == nkb - 1))
                    if kb == nkb - 1:
                        red = small.tile([P, QT], f32, tag="red", name="red")
                        nc.gpsimd.partition_all_reduce(
                            red[:], st["asum"][:], channels=P,
                            reduce_op=bass_isa.ReduceOp.add)
                        rb = small.tile([P, QT], f32, tag="rb", name="rb")
                        nc.vector.reciprocal_approx_fast(rb[:], red[:])
                        nc.vector.tensor_mul(
                            avN[h][:, q0:q0 + QT], st["av"][:], rb[:])
                        del ast[(tq, h)]

                def attn_quarter(tq, inter=()):
                    inter = list(inter)
                    seq = [(h, kb) for h in range(HQ)
                           for kb in range(4 * tq + 4)]
                    attn_scores(tq, seq[0][0], seq[0][1])
                    for i in range(1, len(seq)):
                        attn_scores(tq, *seq[i])
                        attn_av(tq, *seq[i - 1])
                        if seq[i][1] == 0 and inter:
                            # head boundary: slot in o_proj work so the PE
                            # stays hot while the ScalarE catches up on exp
                            inter.pop(0)()
                    attn_av(tq, *seq[-1])
                    for f in inter:
                        f()

                def oproj_tb(tb):
                    ot = ost_p.tile([P, HID], f16, tag="ot", name="ot")
                    for ob in range(4):
                        o_ps = o_pp.tile([P, QT], f32, tag="o", name="o_ps")
                        for hh in range(HQ):
                            nc.tensor.matmul(
                                o_ps[:],
                                lhsT=avN[hh][:, tb * P:(tb + 1) * P],
                                rhs=wo_sb[:, hh, ob * QT:(ob + 1) * QT],
                                start=(hh == 0), stop=(hh == HQ - 1))
                        osl = ot[:, ob * QT:(ob + 1) * QT]
                        if ob % 2 == 0:
                            nc.scalar.copy(osl, o_ps[:])
                        else:
                            nc.vector.tensor_copy(osl, o_ps[:])
                    n = 8 if tb >= 12 else 4
                    step = P // n
                    for i in range(n):
                        nc.sync.dma_start(out_r[i * step:(i + 1) * step, tb, :],
                                          ot[i * step:(i + 1) * step, :])

                def oproj_quarter(tq):
                    for tb in range(4 * tq, 4 * tq + 4):
                        oproj_tb(tb)

                # ---- HAM warmup: keep the PE busy ~3.5us so the clock
                # ramps before the DMA-paced first projections ----
                junk = s_pp.tile([1, QT], f32, tag="s", name="junk")
                for _ in range(64):
                    nc.tensor.matmul(junk[0:1, 0:P], lhsT=ones[:],
                                     rhs=ident[:], start=True, stop=True)

                # ---- streamed schedule ----
                def proj_quarter(tq):
                    proj_kq0(tq)
                    proj_v(tq)
                    for h in range(1, HQ):
                        proj_q(tq, h)

                proj_quarter(0)
                attn_quarter(0)
                proj_quarter(1)
                attn_quarter(1)
                proj_quarter(2)
                oproj_quarter(0)
                proj_quarter(3)
                xt_ctx.close()
                attn_quarter(2, inter=[lambda tb=tb: oproj_tb(tb)
                                       for tb in range(4, 8)])
                attn_quarter(3, inter=[lambda tb=tb: oproj_tb(tb)
                                       for tb in range(8, 12)])
                oproj_quarter(3)

    nc.compile()
    return nc


def _prep_inputs(x, freqs_cis, wq, wk, wv, wo):
    x = np.asarray(x, dtype=np.float32)
    freqs = np.asarray(freqs_cis, dtype=np.float32)
    wq = np.asarray(wq, dtype=np.float32)
    wk = np.asarray(wk, dtype=np.float32)
    wv = np.asarray(wv, dtype=np.float32)
    wo = np.asarray(wo, dtype=np.float32)

    # RoPE row layout: partition 32*qd + i (i<16) = real part of pair
    # 16*qd + i, partition 32*qd + 16 + i = its imag part. Partners sit in
    # the same 32-partition quadrant so DVE stream_shuffle (quadrant-local)
    # can do the half-swap without a DMA.
    pairidx = np.empty(P, dtype=np.int64)
    sign = np.empty(P, dtype=np.float32)
    perm = np.empty(P, dtype=np.int64)
    for qd in range(4):
        for i in range(16):
            pairidx[32 * qd + i] = 16 * qd + i
            pairidx[32 * qd + 16 + i] = 16 * qd + i
            sign[32 * qd + i] = -1.0
            sign[32 * qd + 16 + i] = 1.0
            perm[32 * qd + i] = 2 * (16 * qd + i)
            perm[32 * qd + 16 + i] = 2 * (16 * qd + i) + 1
    cos = freqs[..., 0].T.astype(np.float32)            # [64, S]
    sin = freqs[..., 1].T.astype(np.float32)
    cs2 = np.ascontiguousarray(cos[pairidx, :]).astype(F16)
    ss2 = np.ascontiguousarray(sign[:, None] * sin[pairidx, :]).astype(F16)

    wq_p = (wq.reshape(HID, H, D)[:, :, perm] * D**-0.5).astype(F16)
    wk_p = wk.reshape(HID, KVH, D)[:, :, perm].astype(F16)
    wv_r = wv.reshape(HID, KVH, D).astype(F16)
    wo_r = wo.reshape(H, D, HID)

    kk = np.arange(P)[:, None]
    qq = np.arange(P)[None, :]
    # additive causal mask for the diagonal 128-block: 0 keep, -6e4 drop
    tri = np.where(kk <= qq, 0.0, -60000.0).astype(F16)

    # xT chunk groups: row (tq, kg, p) holds 4 ko-chunks of 512 tokens each
    # (4KB contiguous per partition -> one DMA descriptor per partition)
    xT = x.transpose(0, 2, 1).reshape(B, 4, 4, P, NQ, QT)  # [b,kg,j,p,tq,qt]
    xT = np.ascontiguousarray(xT.transpose(0, 4, 1, 3, 2, 5)).astype(F16)
    xT = xT.reshape(B, NQ * 4 * P, 4 * QT)

    def swz(w):  # [HID, N] -> [P, KO*N] so each partition's DMA is contiguous
        n = w.shape[1]
        return np.ascontiguousarray(
            w.reshape(KO, P, n).transpose(1, 0, 2).reshape(P, KO * n))

    in_maps = []
    for c in range(N_CORES):
        b, g = c // 4, c % 4
        # wq host layout [P, HQ, KO, D]: per-head contiguous for split DMAs
        wq_g = wq_p[:, 4 * g:4 * g + HQ, :]             # [HID, HQ, D]
        wq_sw = wq_g.reshape(KO, P, HQ, D).transpose(1, 2, 0, 3)
        wq_sw = np.ascontiguousarray(wq_sw).reshape(P, HQ * KO * D)
        wo_g = wo_r[4 * g:4 * g + HQ].astype(F16)       # [HQ, P, HID]
        in_maps.append({
            "xT": xT[b],
            "wq": wq_sw,
            "wk": swz(wk_p[:, g, :]),
            "wv": swz(wv_r[:, g, :]),
            "wo": np.ascontiguousarray(
                wo_g.transpose(1, 0, 2).reshape(P, HQ * HID)),
            "cs2": cs2,
            "ss2": ss2,
            "mask": tri,
        })
    return in_maps


def _ensure_ntff_hook():
    """Optional: register the NTFF profiling hook if the image's antenv lacks
    it, so BASS_TRACE=1 produces a profile instead of crashing. No-op on
    failure or when the hook already exists."""
    import sys as _sys
    import types as _types
    try:
        from antenv.axon_hooks import get_axon_ntff_profile_hook  # noqa: F401
        return
    except ImportError:
        pass
    try:
        from trn_agent_boot.trn_boot import _ntff_profile_via_ctypes
        hook = _ntff_profile_via_ctypes("/opt/axon/libaxon_pjrt.so")
        mod = _types.ModuleType("antenv.axon_hooks")
        mod.get_axon_ntff_profile_hook = lambda: hook
        mod.set_axon_ntff_profile_hook = lambda h: None
        _sys.modules["antenv.axon_hooks"] = mod
    except Exception:
        pass


def kernel(x, freqs_cis, wq, wk, wv, wo):
    from concourse.bass_utils import run_bass_kernel_spmd
    _ensure_ntff_hook()

    nc = _CACHE.get("nc")
    if nc is None:
        nc = _build_nc()
        _CACHE["nc"] = nc

    in_maps = _prep_inputs(x, freqs_cis, wq, wk, wv, wo)
    res = run_bass_kernel_spmd(nc, in_maps, list(range(N_CORES)))
    _CACHE["last_result"] = res
    parts = [np.asarray(res.results[c]["out"]).astype(np.float32)
             for c in range(N_CORES)]
    out = np.stack([parts[0] + parts[1] + parts[2] + parts[3],
                    parts[4] + parts[5] + parts[6] + parts[7]])
    return out


# revision 23
# speedup vs baseline: 1.0978x; 1.0978x over previous
"""Causal GQA attention (B=2, S=2048, HID=2048, H=16, KVH=4, D=128) on 8 TRN2 cores.

Sharding: core c -> batch c//4, kv-group c%4 (4 q-heads + 1 kv-head).
o_proj is row-split by head group; host sums the 4 partials per batch.

Device kernel (per core, fp16 matmuls / f32 accumulation), fully streamed:
the sequence is processed in four 512-token quarters. Per quarter: v/k
projections (ko-interleaved so PE starts as soon as each xT chunk lands),
q projections, then attention for that query quarter (kv blocks 0..end of
quarter) and o_proj for the previous quarter are interleaved between later
projections so the PE never sits behind a DMA or phase barrier.

Softmax denominators are NOT computed with ones-matmuls on the PE (that
doubles attention PE traffic); instead the exp tiles are accumulated
elementwise on the DVE (fp16 atsum) and a single [1,512] ones-matmul per
(head, quarter) reduces across partitions.
"""

import numpy as np
import ml_dtypes

F16 = np.float16

B, S, HID = 2, 2048, 2048
H, KVH, D = 16, 4, 128
P = 128
KO = HID // P          # 16 contraction chunks of 128
HQ = H // KVH          # 4 q heads per core
NQ = 4                 # token quarters
QT = S // NQ           # 512 tokens per quarter
NKB = S // P           # 16 key blocks of 128
N_CORES = 8

_CACHE = {}


def _build_nc():
    import concourse.tile as tile
    from concourse import bacc, mybir
    from concourse.masks import make_identity
    import concourse.bass_isa as bass_isa
    from contextlib import ExitStack

    f16 = mybir.dt.float16
    f32 = mybir.dt.float32
    AF = mybir.ActivationFunctionType

    nc = bacc.Bacc("TRN2", target_bir_lowering=False, debug=False,
                   num_devices=N_CORES)

    xT_d = nc.dram_tensor("xT", [NQ * 4 * P, 4 * QT], f16, kind="ExternalInput").ap()
    wq_d = nc.dram_tensor("wq", [P, HQ * KO * D], f16, kind="ExternalInput").ap()
    wk_d = nc.dram_tensor("wk", [P, KO * D], f16, kind="ExternalInput").ap()
    wv_d = nc.dram_tensor("wv", [P, KO * D], f16, kind="ExternalInput").ap()
    wo_d = nc.dram_tensor("wo", [P, HQ * HID], f16, kind="ExternalInput").ap()
    cs_d = nc.dram_tensor("cs2", [P, S], f16, kind="ExternalInput").ap()
    ss_d = nc.dram_tensor("ss2", [P, S], f16, kind="ExternalInput").ap()
    mk_d = nc.dram_tensor("mask", [P, P], f16, kind="ExternalInput").ap()
    out_d = nc.dram_tensor("out", [S, HID], f16, kind="ExternalOutput").ap()
    out_r = out_d.rearrange("(tb p) o -> p tb o", p=P)

    with tile.TileContext(nc) as tc:
        with ExitStack() as octx:
            const = octx.enter_context(tc.tile_pool(name="const", bufs=1))
            rope_p = octx.enter_context(tc.tile_pool(name="rope", bufs=3))
            at_p = octx.enter_context(tc.tile_pool(name="at", bufs=5))
            asum_p = octx.enter_context(tc.tile_pool(name="asum", bufs=2))
            small = octx.enter_context(tc.tile_pool(name="small", bufs=3))
            ost_p = octx.enter_context(tc.tile_pool(name="ost", bufs=4))
            xt_ctx = ExitStack()
            xt_pool = xt_ctx.enter_context(tc.tile_pool(name="xt", bufs=1))

            # ---- persistent input DMAs, emitted in consumption order.
            # The DMA queues are descriptor-rate-bound (~4KB/partition per
            # descriptor is the sweet spot) and one dma_start lands on one
            # queue, so every big tensor is packed into 4KB-per-partition
            # runs and split into partition-halves to parallelize across
            # queues. ----
            def dmaN(dst, src, n):
                step = P // n
                for i in range(n):
                    nc.sync.dma_start(dst[i * step:(i + 1) * step],
                                      src[i * step:(i + 1) * step])

            def dma2(dst, src):
                dmaN(dst, src, 2)

            # first-needed tensors in partition-quarters (4 queues each)
            wk_sb = const.tile([P, KO, D], f16, tag="wk", name="wk")
            dmaN(wk_sb, wk_d.rearrange("p (ko n) -> p ko n", ko=KO), 4)
            wv_sb = const.tile([P, KO, D], f16, tag="wv", name="wv")
            dmaN(wv_sb, wv_d.rearrange("p (ko n) -> p ko n", ko=KO), 4)

            wq_sb = const.tile([P, HQ, KO, D], f16, tag="wq", name="wq")
            wq_r = wq_d.rearrange("p (h ko n) -> p h ko n", h=HQ, ko=KO)
            dmaN(wq_sb[:, 0], wq_r[:, 0], 4)

            # xT arrives as groups of 4 ko-chunks (4KB/partition per DMA)
            xTg = [[None] * 4 for _ in range(NQ)]

            def load_xq(tq, n=2):
                for kg in range(4):
                    t = xt_pool.tile([P, 4, QT], f16, tag=f"xT{tq}_{kg}",
                                     name=f"xT{tq}_{kg}")
                    base = (tq * 4 + kg) * P
                    dmaN(t.rearrange("p a b -> p (a b)"),
                         xT_d[base:base + P, :], n)
                    xTg[tq][kg] = t

            def xchunk(tq, ko):
                return xTg[tq][ko // 4][:, ko % 4, :]

            load_xq(0, n=4)

            for h in range(1, HQ):
                dma2(wq_sb[:, h], wq_r[:, h])

            cs_sb = const.tile([P, S], f16, tag="cs", name="cs")
            dma2(cs_sb, cs_d)
            ss_sb = const.tile([P, S], f16, tag="ss", name="ss")
            dma2(ss_sb, ss_d)
            mk_sb = const.tile([P, P], f16, tag="mk", name="mk")
            nc.sync.dma_start(mk_sb[:], mk_d[:])

            for tq in range(1, NQ):
                load_xq(tq)

            wo_sb = const.tile([P, HQ, HID], f16, tag="wo", name="wo")
            wo_r = wo_d.rearrange("p (h o) -> p h o", h=HQ)
            for h in range(HQ):
                dma2(wo_sb[:, h], wo_r[:, h])

            ident = const.tile([P, P], f16, tag="ident", name="ident")
            make_identity(nc, ident[:])
            ones = const.tile([P, 1], f16, tag="ones", name="ones")
            nc.vector.memset(ones[:], 1.0)

            qR = [const.tile([P, S], f16, tag=f"qR{h}", name=f"qR{h}")
                  for h in range(HQ)]
            kR = const.tile([P, S], f16, tag="kR", name="kR")
            vN = const.tile([P, NKB, D], f16, tag="vN", name="vN")
            avN = [const.tile([P, S], f16, tag=f"avN{h}", name=f"avN{h}")
                   for h in range(HQ)]

            with ExitStack() as ctx:
                # PSUM: proj(2) + s(2) + av(2) + oproj(2) = 8 banks
                proj_pp = ctx.enter_context(
                    tc.tile_pool(name="ps_proj", bufs=2, space="PSUM"))
                s_pp = ctx.enter_context(
                    tc.tile_pool(name="ps_s", bufs=2, space="PSUM"))
                av_pp = ctx.enter_context(
                    tc.tile_pool(name="ps_av", bufs=2, space="PSUM"))
                o_pp = ctx.enter_context(
                    tc.tile_pool(name="ps_o", bufs=2, space="PSUM"))

                # rotate partitions by 64 (stream_shuffle moves 4-partition
                # groups); keeps the rope half-swap off the DMA queues
                SWAP64 = list(range(16, 32)) + list(range(16))

                def rope_tile(ps, out_sl, tsl):
                    raw = rope_p.tile([P, QT], f16, tag="rp_raw", name="rp_raw")
                    nc.scalar.copy(raw[:], ps[:])
                    sw = rope_p.tile([P, QT], f16, tag="rp_sw", name="rp_sw")
                    nc.vector.stream_shuffle(sw[:], raw[:], SWAP64)
                    t1 = rope_p.tile([P, QT], f16, tag="rp_t1", name="rp_t1")
                    nc.vector.tensor_mul(t1[:], raw[:], cs_sb[:, tsl])
                    t2 = rope_p.tile([P, QT], f16, tag="rp_t2", name="rp_t2")
                    nc.vector.tensor_mul(t2[:], sw[:], ss_sb[:, tsl])
                    nc.vector.tensor_add(out_sl, t1[:], t2[:])

                def proj_kq0(tq):
                    """k and q-head-0 projections, ko-interleaved so the PE
                    starts on each xT chunk as soon as its DMA lands. Each
                    GEMM accumulates in its own PSUM bank — hardware PSUM
                    accumulation breaks if two start=True groups share a
                    bank."""
                    k_ps = proj_pp.tile([P, QT], f32, tag="proj", name="k_ps")
                    q_ps = proj_pp.tile([P, QT], f32, tag="proj", name="q_ps")
                    for ko in range(KO):
                        st, sp = ko == 0, ko == KO - 1
                        x = xchunk(tq, ko)
                        nc.tensor.matmul(k_ps[:], lhsT=wk_sb[:, ko, :],
                                         rhs=x, start=st, stop=sp)
                        nc.tensor.matmul(q_ps[:], lhsT=wq_sb[:, 0, ko, :],
                                         rhs=x, start=st, stop=sp)
                    tsl = slice(tq * QT, (tq + 1) * QT)
                    rope_tile(k_ps, kR[:, tsl], tsl)
                    rope_tile(q_ps, qR[0][:, tsl], tsl)

                def proj_v(tq):
                    """v projection straight to natural [tok, d] layout:
                    lhsT = xT chunk block, rhs = wv. The four 128-token
                    blocks run sequentially, each in its own psum ring slot
                    (one open accumulation group per bank)."""
                    for i in range(4):
                        v_ps = proj_pp.tile([P, P], f32, tag="proj",
                                            name="v_ps")
                        for ko in range(KO):
                            nc.tensor.matmul(
                                v_ps[:],
                                lhsT=xchunk(tq, ko)[:, i * P:(i + 1) * P],
                                rhs=wv_sb[:, ko, :],
                                start=(ko == 0), stop=(ko == KO - 1))
                        nc.scalar.copy(vN[:, 4 * tq + i, :], v_ps[:])

                def proj_q(tq, h):
                    q_ps = proj_pp.tile([P, QT], f32, tag="proj", name="q_ps")
                    for ko in range(KO):
                        nc.tensor.matmul(
                            q_ps[:], lhsT=wq_sb[:, h, ko, :],
                            rhs=xchunk(tq, ko),
                            start=(ko == 0), stop=(ko == KO - 1))
                    tsl = slice(tq * QT, (tq + 1) * QT)
                    rope_tile(q_ps, qR[h][:, tsl], tsl)

                # ---- attention for query quarter tq, head h ----
                # per-kb: scores -> exp -> (tri-mask) -> {AV matmul, atsum}
                # issue scores[i+1] before AV[i] so the PE always has work
                # while the ScalarE exps the previous block.
                ast = {}

                def attn_scores(tq, h, kb):
                    nkb = 4 * tq + 4
                    q0 = tq * QT
                    if kb == 0:
                        ast[(tq, h)] = {
                            "av": av_pp.tile([P, QT], f32, tag="av", name="av"),
                            "asum": asum_p.tile([P, QT], f16, tag="asum",
                                                name="asum"),
                            "ats": {},
                        }
                    st = ast[(tq, h)]
                    r = kb - 4 * tq
                    lo = P * r if r >= 0 else 0
                    s_ps = s_pp.tile([P, QT], f32, tag="s", name="s")
                    diag = r >= 0
                    nc.tensor.matmul(
                        s_ps[:, lo:QT],
                        lhsT=kR[:, kb * P:(kb + 1) * P],
                        rhs=qR[h][:, q0 + lo:q0 + QT],
                        start=True, stop=not diag)
                    if diag:
                        # push masked entries to -inf inside the psum group:
                        # exp then underflows to 0, no DVE hop before AV
                        nc.tensor.matmul(
                            s_ps[:, lo:lo + P], lhsT=ident[:], rhs=mk_sb[:],
                            start=False, stop=True)
                    at = at_p.tile([P, QT], f16, tag="at", name="at")
                    nc.scalar.activation(at[:, lo:QT], s_ps[:, lo:QT], AF.Exp)
                    if kb == 0:
                        nc.vector.tensor_copy(st["asum"][:], at[:])
                    else:
                        nc.vector.tensor_add(st["asum"][:, lo:QT],
                                             st["asum"][:, lo:QT], at[:, lo:QT])
                    st["ats"][kb] = at

                def attn_av(tq, h, kb):
                    nkb = 4 * tq + 4
                    q0 = tq * QT
                    st = ast[(tq, h)]
                    r = kb - 4 * tq
                    lo = P * r if r >= 0 else 0
                    at = st["ats"].pop(kb)
                    nc.tensor.matmul(
                        st["av"][:, lo:QT], lhsT=vN[:, kb, :],
                        rhs=at[:, lo:QT],
                        start=(kb == 0), stop=(kb == nkb - 1))
                    if kb == nkb - 1:
                        dn_ps = s_pp.tile([1, QT], f32, tag="s", name="dn_ps")
                        nc.tensor.matmul(dn_ps[0:1, :], lhsT=ones[:],
                                         rhs=st["asum"][:], start=True,
                                         stop=True)
                        rc = small.tile([1, QT], f32, tag="rc", name="rc")
                        nc.vector.reciprocal_approx_fast(rc[:], dn_ps[:])
                        rb = small.tile([P, QT], f32, tag="rb", name="rb")
                        nc.gpsimd.partition_broadcast(rb[:], rc[:])
                        nc.vector.tensor_mul(
                            avN[h][:, q0:q0 + QT], st["av"][:], rb[:])
                        del ast[(tq, h)]

                def attn_quarter(tq, inter=()):
                    inter = list(inter)
                    seq = [(h, kb) for h in range(HQ)
                           for kb in range(4 * tq + 4)]
                    attn_scores(tq, seq[0][0], seq[0][1])
                    for i in range(1, len(seq)):
                        attn_scores(tq, *seq[i])
                        attn_av(tq, *seq[i - 1])
                        if seq[i][1] == 0 and inter:
                            # head boundary: slot in o_proj work so the PE
                            # stays hot while the ScalarE catches up on exp
                            inter.pop(0)()
                    attn_av(tq, *seq[-1])
                    for f in inter:
                        f()

                def oproj_tb(tb):
                    ot = ost_p.tile([P, HID], f16, tag="ot", name="ot")
                    for ob in range(4):
                        o_ps = o_pp.tile([P, QT], f32, tag="o", name="o_ps")
                        for hh in range(HQ):
                            nc.tensor.matmul(
                                o_ps[:],
                                lhsT=avN[hh][:, tb * P:(tb + 1) * P],
                                rhs=wo_sb[:, hh, ob * QT:(ob + 1) * QT],
                                start=(hh == 0), stop=(hh == HQ - 1))
                        osl = ot[:, ob * QT:(ob + 1) * QT]
                        if ob % 2 == 0:
                            nc.scalar.copy(osl, o_ps[:])
                        else:
                            nc.vector.tensor_copy(osl, o_ps[:])
                    n = 8 if tb >= 12 else 4
                    step = P // n
                    for i in range(n):
                        nc.sync.dma_start(out_r[i * step:(i + 1) * step, tb, :],
                                          ot[i * step:(i + 1) * step, :])

                def oproj_quarter(tq):
                    for tb in range(4 * tq, 4 * tq + 4):
                        oproj_tb(tb)

                # ---- HAM warmup: keep the PE busy ~3.5us so the clock
                # ramps before the DMA-paced first projections ----
                junk = s_pp.tile([1, QT], f32, tag="s", name="junk")
                for _ in range(64):
                    nc.tensor.matmul(junk[0:1, 0:P], lhsT=ones[:],
                                     rhs=ident[:], start=True, stop=True)

                # ---- streamed schedule ----
                def proj_quarter(tq):
                    proj_kq0(tq)
                    proj_v(tq)
                    for h in range(1, HQ):
                        proj_q(tq, h)

                proj_quarter(0)
                attn_quarter(0)
                proj_quarter(1)
                attn_quarter(1)
                proj_quarter(2)
                oproj_quarter(0)
                proj_quarter(3)
                xt_ctx.close()
                attn_quarter(2, inter=[lambda tb=tb: oproj_tb(tb)
                                       for tb in range(4, 8)])
                attn_quarter(3, inter=[lambda tb=tb: oproj_tb(tb)
                                       for tb in range(8, 12)])
                oproj_quarter(3)

    nc.compile()
    return nc


def _prep_inputs(x, freqs_cis, wq, wk, wv, wo):
    x = np.asarray(x, dtype=np.float32)
    freqs = np.asarray(freqs_cis, dtype=np.float32)
    wq = np.asarray(wq, dtype=np.float32)
    wk = np.asarray(wk, dtype=np.float32)
    wv = np.asarray(wv, dtype=np.float32)
    wo = np.asarray(wo, dtype=np.float32)

    # RoPE row layout: partition 32*qd + i (i<16) = real part of pair
    # 16*qd + i, partition 32*qd + 16 + i = its imag part. Partners sit in
    # the same 32-partition quadrant so DVE stream_shuffle (quadrant-local)
    # can do the half-swap without a DMA.
    pairidx = np.empty(P, dtype=np.int64)
    sign = np.empty(P, dtype=np.float32)
    perm = np.empty(P, dtype=np.int64)
    for qd in range(4):
        for i in range(16):
            pairidx[32 * qd + i] = 16 * qd + i
            pairidx[32 * qd + 16 + i] = 16 * qd + i
            sign[32 * qd + i] = -1.0
            sign[32 * qd + 16 + i] = 1.0
            perm[32 * qd + i] = 2 * (16 * qd + i)
            perm[32 * qd + 16 + i] = 2 * (16 * qd + i) + 1
    cos = freqs[..., 0].T.astype(np.float32)            # [64, S]
    sin = freqs[..., 1].T.astype(np.float32)
    cs2 = np.ascontiguousarray(cos[pairidx, :]).astype(F16)
    ss2 = np.ascontiguousarray(sign[:, None] * sin[pairidx, :]).astype(F16)

    wq_p = (wq.reshape(HID, H, D)[:, :, perm] * D**-0.5).astype(F16)
    wk_p = wk.reshape(HID, KVH, D)[:, :, perm].astype(F16)
    wv_r = wv.reshape(HID, KVH, D).astype(F16)
    wo_r = wo.reshape(H, D, HID)

    kk = np.arange(P)[:, None]
    qq = np.arange(P)[None, :]
    # additive causal mask for the diagonal 128-block: 0 keep, -6e4 drop
    tri = np.where(kk <= qq, 0.0, -60000.0).astype(F16)

    # xT chunk groups: row (tq, kg, p) holds 4 ko-chunks of 512 tokens each
    # (4KB contiguous per partition -> one DMA descriptor per partition)
    xT = x.transpose(0, 2, 1).reshape(B, 4, 4, P, NQ, QT)  # [b,kg,j,p,tq,qt]
    xT = np.ascontiguousarray(xT.transpose(0, 4, 1, 3, 2, 5)).astype(F16)
    xT = xT.reshape(B, NQ * 4 * P, 4 * QT)

    def swz(w):  # [HID, N] -> [P, KO*N] so each partition's DMA is contiguous
        n = w.shape[1]
        return np.ascontiguousarray(
            w.reshape(KO, P, n).transpose(1, 0, 2).reshape(P, KO * n))

    in_maps = []
    for c in range(N_CORES):
        b, g = c // 4, c % 4
        # wq host layout [P, HQ, KO, D]: per-head contiguous for split DMAs
        wq_g = wq_p[:, 4 * g:4 * g + HQ, :]             # [HID, HQ, D]
        wq_sw = wq_g.reshape(KO, P, HQ, D).transpose(1, 2, 0, 3)
        wq_sw = np.ascontiguousarray(wq_sw).reshape(P, HQ * KO * D)
        wo_g = wo_r[4 * g:4 * g + HQ].astype(F16)       # [HQ, P, HID]
        in_maps.append({
            "xT": xT[b],
            "wq": wq_sw,
            "wk": swz(wk_p[:, g, :]),
            "wv": swz(wv_r[:, g, :]),
            "wo": np.ascontiguousarray(
                wo_g.transpose(1, 0, 2).reshape(P, HQ * HID)),
            "cs2": cs2,
            "ss2": ss2,
            "mask": tri,
        })
    return in_maps


def _ensure_ntff_hook():
    """Optional: register the NTFF profiling hook if the image's antenv lacks
    it, so BASS_TRACE=1 produces a profile instead of crashing. No-op on
    failure or when the hook already exists."""
    import sys as _sys
    import types as _types
    try:
        from antenv.axon_hooks import get_axon_ntff_profile_hook  # noqa: F401
        return
    except ImportError:
        pass
    try:
        from trn_agent_boot.trn_boot import _ntff_profile_via_ctypes
        hook = _ntff_profile_via_ctypes("/opt/axon/libaxon_pjrt.so")
        mod = _types.ModuleType("antenv.axon_hooks")
        mod.get_axon_ntff_profile_hook = lambda: hook
        mod.set_axon_ntff_profile_hook = lambda h: None
        _sys.modules["antenv.axon_hooks"] = mod
    except Exception:
        pass


def kernel(x, freqs_cis, wq, wk, wv, wo):
    from concourse.bass_utils import run_bass_kernel_spmd
    _ensure_ntff_hook()

    nc = _CACHE.get("nc")
    if nc is None:
        nc = _build_nc()
        _CACHE["nc"] = nc

    in_maps = _prep_inputs(x, freqs_cis, wq, wk, wv, wo)
    res = run_bass_kernel_spmd(nc, in_maps, list(range(N_CORES)))
    _CACHE["last_result"] = res
    parts = [np.asarray(res.results[c]["out"]).astype(np.float32)
             for c in range(N_CORES)]
    out = np.stack([parts[0] + parts[1] + parts[2] + parts[3],
                    parts[4] + parts[5] + parts[6] + parts[7]])
    return out
